# revision 35
# baseline (speedup 1.0000x reference)
"""Trainium2 Bass kernel for AdaptiveURBFLayer.

Computation:  out[b, j] = coefs[j] * exp(-0.5 * ((xe[b,j]-means[j])/vars_[j])^2)
where xe = x @ expansion_mapping.T and expansion_mapping is a (scaled) one-hot
row matrix: out feature j reads exactly one input feature idx[j].

Strategy (8 NeuronCores, data-parallel over batch):
  - Host extracts idx[j] = argmax(em[j]) and the scale em[j, idx[j]]; out
    features are grouped by source input feature (16 per feature here).
  - Device layout puts input features on SBUF partitions: x^T tiles
    [128, B_CORE].  Every group-of-16 output feature then has its RBF params
    as per-partition scalars.
  - Fast path (means folded to zero bias): out = exp(alpha*x^2 + lnc) with
    alpha = -0.5*(scale/vars)^2 per partition:
        DVE:  x2 = x*x (once per input tile)
              y  = x2*alpha + lnc   (tensor_scalar, 2x fp32 mode)
        ACT:  g  = Exp(y)           (grouped over 4 output units)
        DMA:  g -> DRAM (contiguous 2MB blocks)
  - Output is written feature-major per core; host transposes back.
"""

import sys
import types

import numpy as np


def _install_ntff_shim():
    """Make antenv.axon_hooks importable so run_bass_kernel_spmd(trace=True)
    can capture NTFF profiles under axon. Harmless if unavailable."""
    try:
        from antenv.axon_hooks import get_axon_ntff_profile_hook  # noqa: F401
        return
    except ImportError:
        pass
    try:
        from trn_agent_boot.trn_boot import _ntff_profile_via_ctypes
        hook = _ntff_profile_via_ctypes("/opt/axon/libaxon_pjrt.so")
    except Exception:
        hook = None
    m = types.ModuleType("antenv.axon_hooks")
    m.get_axon_ntff_profile_hook = lambda: hook
    m.set_axon_ntff_profile_hook = lambda h: None
    sys.modules["antenv.axon_hooks"] = m


_install_ntff_shim()

from concourse import bass, mybir, tile  # noqa: E402
from concourse.bass_utils import run_bass_kernel_spmd  # noqa: E402
from concourse.vector_clock import ScopedClock  # noqa: E402

N_CORES = 8
F32 = mybir.dt.float32


# ---------------------------------------------------------------------------
# Compat patches: this walrus build allows at most ONE sync-wait per
# instruction.  (1) split the Tile tail-drain waits over several drains;
# (2) post-pass that rewrites any multi-wait instruction into single-wait
# NoOps (same engine, immediately before) + the instruction with one wait.
# ---------------------------------------------------------------------------

# Tail style knob: "full" = drain + barrier + sem clear + barrier (safe);
# "slim" = drain + single barrier, no sem clears (the NEFF postamble zeroes
# every semaphore anyway — observed in traces).
TAIL_STYLE = "slim"

# Fast-path schedule: (n_units, issuing_engine) per group, covering all
# t_tiles*k units in order.  Small groups at the ends prime the DMA pipeline
# early and shrink the final drain; big groups amortize instruction overhead
# mid-stream.  'sync' and 'scalar' alternate so both HWDGE queues share the
# bytes (~300 B/ns per-queue cap; ~570 aggregate).
PLAN = [
    (2, "sync"), (2, "scalar"),
    (4, "sync"), (4, "scalar"), (4, "sync"), (4, "scalar"), (4, "scalar"),
    (2, "sync"), (2, "scalar"), (2, "sync"), (2, "sync"),
]


def _patched_drain_and_barrier(self, tick_clock, wait_clock):
    drain_inst = self.nc.sync.drain()
    wait_clock.add_sem_waits(
        drain_inst.ins, ScopedClock({None: tick_clock.global_clock})
    )
    si = drain_inst.ins.sync_info
    waits = list(si.on_wait) if si is not None else []
    if len(waits) > 1:
        drain_inst.ins.sync_info = mybir.SyncInfo(
            on_wait=waits[:1], on_update=list(si.on_update)
        )
        for w in waits[1:]:
            extra = self.nc.sync.drain()
            extra.ins.sync_info = mybir.SyncInfo(on_wait=[w], on_update=[])
    assert self.sems is not None
    popped = self.nc._tile_sem_poison_stack.pop()
    assert popped is self._sem_poison
    if TAIL_STYLE == "full":
        self.nc.all_engine_barrier()
        self.nc.clear_and_free_semaphores(list(self.sems.allocated().values()))
        self.nc.all_engine_barrier()
    else:
        self.nc.all_engine_barrier()


tile.TileContext._drain_and_barrier = _patched_drain_and_barrier


def _split_multi_waits(nc):
    """Rewrite every instruction carrying >1 sem wait: extra waits move onto
    fresh single-wait NoOps on the same engine placed immediately before."""
    eng_by_type = {
        mybir.EngineType.SP: nc.sync,
        mybir.EngineType.Activation: nc.scalar,
        mybir.EngineType.DVE: nc.vector,
        mybir.EngineType.Pool: nc.gpsimd,
        mybir.EngineType.PE: nc.tensor,
    }
    for fn in nc.m.functions:
        for bb in fn.blocks:
            insts = list(bb.instructions)
            if not any(
                i.sync_info is not None and len(i.sync_info.on_wait) > 1
                for i in insts
            ):
                continue
            new_list = []
            for inst in insts:
                si = inst.sync_info
                if si is not None and len(si.on_wait) > 1:
                    waits = list(si.on_wait)
                    for w in waits[:-1]:
                        nop = eng_by_type[inst.engine].nop(nofuse=True)
                        # nop() appended itself to the current bb; steal it.
                        cur = nc.cur_bb.bb
                        assert cur.instructions[-1] is nop.ins
                        cur.instructions.pop()
                        nop.ins.sync_info = mybir.SyncInfo(
                            on_wait=[w], on_update=[]
                        )
                        new_list.append(nop.ins)
                    inst.sync_info = mybir.SyncInfo(
                        on_wait=[waits[-1]], on_update=list(si.on_update)
                    )
                new_list.append(inst)
            bb.instructions[:] = new_list


# ---------------------------------------------------------------------------
# Device kernel builder
# ---------------------------------------------------------------------------

def _build_nc_fast(in_f, out_f, b_core, plan):
    """Fast path: out = Exp(alpha * x^2 + lnc), alpha/lnc per-partition.

    Per plan group of n units:
      DVE: y[:, u] = x2 * alpha[col] + lnc[col]   (tensor_scalar, 2x fp32)
      ACT: g = Exp(y)                             (one grouped instruction)
      DMA: g -> out pair-blocks                   (SP or ACT HWDGE queue)

    DRAM out layout: pair-blocks [n_units_total//2, 128, 2*b_core]; global
    unit u = t*k + r lives in block u//2, half u%2.
    """
    k = out_f // in_f
    t_tiles = in_f // 128
    ncols = t_tiles * k
    assert sum(n for n, _ in plan) == ncols
    assert all(n % 2 == 0 for n, _ in plan)

    nc = bass.Bass()
    xt_h = nc.declare_dram_parameter("xt", [t_tiles, 128, b_core], F32, isOutput=False)
    # p0 (alpha) and plc (lnc) merged: [128, 2*ncols], alpha cols then lnc cols
    pp_h = nc.declare_dram_parameter("pp", [128, 2 * ncols], F32, isOutput=False)
    out_h = nc.declare_dram_parameter(
        "out", [ncols // 2, 128, 2 * b_core], F32, isOutput=True
    )

    with tile.TileContext(nc) as tc:
        with (
            tc.tile_pool(name="params", bufs=1) as ppool,
            tc.tile_pool(name="xin", bufs=2) as xpool,
            tc.tile_pool(name="xsq", bufs=2) as x2pool,
            tc.tile_pool(name="ybuf", bufs=5) as ypool,
            tc.tile_pool(name="gbuf", bufs=5) as gpool,
        ):
            # params ride the GpSimd SWDGE queue: lands early, off the
            # critical SP queue
            pp = ppool.tile([128, 2 * ncols], F32)
            nc.gpsimd.dma_start(out=pp[:], in_=pp_h[:])

            xs = []
            # x0 in piece-DMAs on SP (whose queue is idle until the first
            # group is produced anyway); later x tiles ride the GpSimd
            # SWDGE queue (plenty of lead time)
            qtr = b_core // 4
            h = b_core // 2
            x0 = xpool.tile([128, b_core], F32, tag="x0")
            nc.sync.dma_start(out=x0[:, :qtr], in_=xt_h[0][:, :qtr])
            nc.sync.dma_start(out=x0[:, qtr:h], in_=xt_h[0][:, qtr:h])
            nc.sync.dma_start(out=x0[:, h:], in_=xt_h[0][:, h:])
            xs.append(x0)
            for t in range(1, t_tiles):
                xt = xpool.tile([128, b_core], F32, tag=f"x{t}")
                nc.gpsimd.dma_start(out=xt[:], in_=xt_h[t])
                xs.append(xt)

            def alpha(col):
                return pp[:, col : col + 1]

            def lnc(col):
                return pp[:, ncols + col : ncols + col + 1]

            x2s = {}

            def get_x2(t):
                if t not in x2s:
                    x2 = x2pool.tile([128, b_core], F32, tag=f"x2_{t}")
                    nc.vector.tensor_tensor(
                        x2[:], xs[t][:], xs[t][:], mybir.AluOpType.mult
                    )
                    x2s[t] = x2
                return x2s[t]

            def emit_group(u0, nu, eng_name, bsl=None):
                """Emit DVE ts + ACT exp + DMA for units [u0, u0+nu).
                bsl: optional (b_lo, b_hi) batch sub-range (priming)."""
                b_lo, b_hi = bsl if bsl else (0, b_core)
                bn = b_hi - b_lo
                key = (u0, nu)
                if key in gtiles:
                    g, y = gtiles[key]
                else:
                    g = gpool.tile([128, nu * b_core], F32, tag="g")
                    y = ypool.tile([128, nu * b_core], F32, tag="y")
                    gtiles[key] = (g, y)
                for j in range(nu):
                    col = u0 + j
                    x2 = get_x2(col // k)
                    sl = slice(j * b_core + b_lo, j * b_core + b_hi)
                    nc.vector.tensor_scalar(
                        y[:, sl], x2[:, b_lo:b_hi], alpha(col), lnc(col),
                        mybir.AluOpType.mult, mybir.AluOpType.add,
                    )
                if bsl is None:
                    y_ap, g_ap = y[:], g[:]
                    dram_ap = out_h[u0 // 2 : (u0 + nu) // 2].rearrange(
                        "c p f -> p c f"
                    )
                else:
                    y_ap = y[:].rearrange("p (j b) -> p j b", j=nu)[
                        :, :, b_lo:b_hi
                    ]
                    g_ap = g[:].rearrange("p (j b) -> p j b", j=nu)[
                        :, :, b_lo:b_hi
                    ]
                    dram_ap = out_h[u0 // 2 : (u0 + nu) // 2].rearrange(
                        "c p (j b) -> p (c j) b", j=2
                    )[:, :, b_lo:b_hi]
                nc.scalar.activation(
                    g_ap, y_ap, mybir.ActivationFunctionType.Exp
                )
                getattr(nc, eng_name).dma_start(out=dram_ap, in_=g_ap)

            gtiles = {}
            start_gi, u0 = 0, 0
            prime = len(plan) >= 2 and plan[0][0] == 2 and plan[1][0] == 2
            if prime:
                # prime the pipeline: first two groups computed and shipped
                # in batch quarters/halves, starting on x0's first piece
                x2_0 = x2pool.tile([128, b_core], F32, tag="x2_0")
                x2s[0] = x2_0
                for lo, hi in ((0, qtr), (qtr, h), (h, b_core)):
                    nc.vector.tensor_tensor(
                        x2_0[:, lo:hi], x0[:, lo:hi], x0[:, lo:hi],
                        mybir.AluOpType.mult,
                    )
                    emit_group(0, 2, plan[0][1], bsl=(lo, hi))
                    if lo == qtr:
                        emit_group(2, 2, plan[1][1], bsl=(0, h))
                emit_group(2, 2, plan[1][1], bsl=(h, b_core))
                if t_tiles > 1:
                    get_x2(1)
                start_gi, u0 = 2, 4
            for gi in range(start_gi, len(plan)):
                nu, eng_name = plan[gi]
                emit_group(u0, nu, eng_name)
                if gi == 0 and t_tiles > 1:
                    get_x2(1)
                u0 += nu

    _split_multi_waits(nc)
    return nc


def _build_nc(in_f, out_f, b_core, mode, group_u=4):
    """mode: 'general' -> out = Exp(-Square(a*x + b) + lnc) params: p0, p1, plc
             'cmul'    -> general + extra per-partition multiply by p2
    """
    k = out_f // in_f          # out features per input feature
    t_tiles = in_f // 128      # input-feature tiles
    n_groups = k // group_u
    ncols = t_tiles * k

    nc = bass.Bass()
    xt_h = nc.declare_dram_parameter("xt", [t_tiles, 128, b_core], F32, isOutput=False)
    p0_h = nc.declare_dram_parameter("p0", [128, ncols], F32, isOutput=False)
    plc_h = nc.declare_dram_parameter("plc", [128, ncols], F32, isOutput=False)
    if mode in ("general", "cmul"):
        p1_h = nc.declare_dram_parameter("p1", [128, ncols], F32, isOutput=False)
    if mode == "cmul":
        p2_h = nc.declare_dram_parameter("p2", [128, ncols], F32, isOutput=False)
    out_h = nc.declare_dram_parameter(
        "out", [t_tiles, n_groups, 128, group_u * b_core], F32, isOutput=True
    )

    with tile.TileContext(nc) as tc:
        with (
            tc.tile_pool(name="params", bufs=1) as ppool,
            tc.tile_pool(name="xin", bufs=2) as xpool,
            tc.tile_pool(name="xsq", bufs=2) as x2pool,
            tc.tile_pool(name="ybuf", bufs=3) as ypool,
            tc.tile_pool(name="gbuf", bufs=4) as gpool,
        ):
            # x tiles first: they head the critical chain to the first
            # output DMA; params load in parallel on other queues.
            xs = []
            for t in range(t_tiles):
                x = xpool.tile([128, b_core], F32, tag=f"x{t}")
                nc.sync.dma_start(out=x[:], in_=xt_h[t])
                xs.append(x)
            p0 = ppool.tile([128, ncols], F32)
            nc.sync.dma_start(out=p0[:], in_=p0_h[:])
            plc = ppool.tile([128, ncols], F32)
            nc.sync.dma_start(out=plc[:], in_=plc_h[:])
            if mode in ("general", "cmul"):
                p1 = ppool.tile([128, ncols], F32)
                nc.sync.dma_start(out=p1[:], in_=p1_h[:])
            if mode == "cmul":
                p2 = ppool.tile([128, ncols], F32)
                nc.sync.dma_start(out=p2[:], in_=p2_h[:])

            for t in range(t_tiles):
                x = xs[t]
                for q in range(n_groups):
                    g = gpool.tile([128, group_u * b_core], F32)
                    y = ypool.tile([128, group_u * b_core], F32)
                    for u in range(group_u):
                        col = t * k + q * group_u + u
                        sl = slice(u * b_core, (u + 1) * b_core)
                        # s = (a*x + b)^2  (per-partition a, b)
                        nc.scalar.activation(
                            y[:, sl],
                            x[:],
                            mybir.ActivationFunctionType.Square,
                            bias=p1[:, col : col + 1],
                            scale=p0[:, col : col + 1],
                        )
                        # g = exp(-s + lnc)
                        nc.scalar.activation(
                            g[:, sl],
                            y[:, sl],
                            mybir.ActivationFunctionType.Exp,
                            bias=plc[:, col : col + 1],
                            scale=-1.0,
                        )
                        if mode == "cmul":
                            nc.vector.tensor_scalar(
                                g[:, sl],
                                g[:, sl],
                                p2[:, col : col + 1],
                                None,
                                mybir.AluOpType.mult,
                            )
                    nc.sync.dma_start(out=out_h[t, q], in_=g[:])

    _split_multi_waits(nc)
    return nc


# ---------------------------------------------------------------------------
# Host orchestration
# ---------------------------------------------------------------------------

def _prep(x, expansion_mapping, means, vars_, coefs):
    x = np.asarray(x, dtype=np.float32)
    em = np.asarray(expansion_mapping, dtype=np.float32)
    means = np.asarray(means, dtype=np.float32)
    vars_ = np.asarray(vars_, dtype=np.float32)
    coefs = np.asarray(coefs, dtype=np.float32)
    return x, em, means, vars_, coefs


def _numpy_fallback(x, em, means, vars_, coefs):
    xe = x @ em.T
    z = (xe - means) / vars_
    return (np.exp(-0.5 * z * z) * coefs).astype(np.float32)


def _run(inputs, trace=False):
    x, em, means, vars_, coefs = _prep(
        inputs["x"], inputs["expansion_mapping"], inputs["means"],
        inputs["vars_"], inputs["coefs"],
    )
    batch, in_f = x.shape
    out_f = em.shape[0]

    idx = em.argmax(axis=1)
    scale_em = em[np.arange(out_f), idx]
    counts = np.bincount(idx, minlength=in_f)
    k = out_f // in_f
    structured = (
        batch % N_CORES == 0
        and in_f % 128 == 0
        and out_f % in_f == 0
        and np.all(counts == k)
        and k % 2 == 0
        # each row must be one-hot (a single nonzero)
        and np.count_nonzero(em) == out_f
    )
    if not structured:
        return _numpy_fallback(x, em, means, vars_, coefs), None

    b_core = batch // N_CORES
    t_tiles = in_f // 128
    ncols = t_tiles * k

    order = np.argsort(idx, kind="stable")  # device row m -> out feature
    a = scale_em[order] / vars_[order]
    b = -means[order] / vars_[order]
    c = coefs[order]

    fast = bool(np.all(b == 0.0))
    if np.all(c >= 0.0):
        with np.errstate(divide="ignore"):
            lc = np.log(c).astype(np.float32)
        cmul = None
    else:
        lc = np.zeros(out_f, dtype=np.float32)
        cmul = c

    if fast and cmul is None:
        mode = "fast"
        p0 = (-0.5 * a * a).astype(np.float32)
    elif cmul is None:
        mode = "general"
        p0 = (a / np.sqrt(2.0)).astype(np.float32)
        p1 = (b / np.sqrt(2.0)).astype(np.float32)
    else:
        mode = "cmul"
        p0 = (a / np.sqrt(2.0)).astype(np.float32)
        p1 = (b / np.sqrt(2.0)).astype(np.float32)

    def dev_layout(p):
        # full [out_f] ordered by device row m = 16*i + r -> [128, ncols]
        return np.ascontiguousarray(
            p.reshape(t_tiles, 128, k).transpose(1, 0, 2).reshape(128, ncols)
        )

    if mode == "fast":
        if ncols == sum(n for n, _ in PLAN):
            plan = PLAN
        else:
            plan = [
                (2, ("sync", "scalar")[i % 2]) for i in range(ncols // 2)
            ]
        nc = _build_nc_fast(in_f, out_f, b_core, plan)
        group_u = n_groups = None
    else:
        group_u = 4 if k % 4 == 0 else k
        n_groups = k // group_u
        nc = _build_nc(in_f, out_f, b_core, mode, group_u)

    xT = np.ascontiguousarray(x.T)  # [in_f, batch]
    if mode == "fast":
        base = {
            "pp": np.ascontiguousarray(
                np.hstack([dev_layout(p0), dev_layout(lc)])
            )
        }
    else:
        base = {"p0": dev_layout(p0), "plc": dev_layout(lc)}
        base["p1"] = dev_layout(p1)
        if mode == "cmul":
            base["p2"] = dev_layout(cmul)
    in_maps = []
    for kcore in range(N_CORES):
        shard = np.ascontiguousarray(
            xT[:, kcore * b_core : (kcore + 1) * b_core]
        ).reshape(t_tiles, 128, b_core)
        in_maps.append({"xt": shard, **base})

    res = run_bass_kernel_spmd(
        nc, in_maps, core_ids=list(range(N_CORES)), trace=trace
    )

    identity_order = bool(np.array_equal(order, np.arange(out_f)))
    out = np.empty((batch, out_f), dtype=np.float32)
    for kcore in range(N_CORES):
        dev = res.results[kcore]["out"]
        if mode == "fast":
            # [ncols//2, 128, 2*b_core]: block = global unit pair
            dev = dev.reshape(t_tiles, k // 2, 128, 2, b_core)
        else:
            # [t, q, 128, group_u*b_core]
            dev = dev.reshape(t_tiles, n_groups, 128, group_u, b_core)
        # row m = 16*(128*t + p) + r  with r = (pairing axes 1,3)
        dev = dev.transpose(0, 2, 1, 3, 4).reshape(out_f, b_core)
        sl = slice(kcore * b_core, (kcore + 1) * b_core)
        if identity_order:
            out[sl] = dev.T
        else:
            out[sl.start : sl.stop, order] = dev.T
    return out, res


def kernel(**inputs):
    out, _ = _run(inputs, trace=False)
    return out


# revision 39
# speedup vs baseline: 1.2899x; 1.2899x over previous
"""Trainium2 Bass kernel for AdaptiveURBFLayer.

Computation:  out[b, j] = coefs[j] * exp(-0.5 * ((xe[b,j]-means[j])/vars_[j])^2)
where xe = x @ expansion_mapping.T and expansion_mapping is a (scaled) one-hot
row matrix: out feature j reads exactly one input feature idx[j].

Strategy (8 NeuronCores, data-parallel over batch):
  - Host extracts idx[j] = argmax(em[j]) and the scale em[j, idx[j]]; out
    features are grouped by source input feature (16 per feature here).
  - Device layout puts input features on SBUF partitions: x^T tiles
    [128, B_CORE].  Every group-of-16 output feature then has its RBF params
    as per-partition scalars.
  - Fast path (means folded to zero bias): out = exp(alpha*x^2 + lnc) with
    alpha = -0.5*(scale/vars)^2 per partition:
        DVE:  x2 = x*x (once per input tile)
              y  = x2*alpha + lnc   (tensor_scalar, 2x fp32 mode)
        ACT:  g  = Exp(y)           (grouped over 4 output units)
        DMA:  g -> DRAM (contiguous 2MB blocks)
  - Output is written feature-major per core; host transposes back.
"""

import sys
import types

import numpy as np


def _install_ntff_shim():
    """Make antenv.axon_hooks importable so run_bass_kernel_spmd(trace=True)
    can capture NTFF profiles under axon. Harmless if unavailable."""
    try:
        from antenv.axon_hooks import get_axon_ntff_profile_hook  # noqa: F401
        return
    except ImportError:
        pass
    try:
        from trn_agent_boot.trn_boot import _ntff_profile_via_ctypes
        hook = _ntff_profile_via_ctypes("/opt/axon/libaxon_pjrt.so")
    except Exception:
        hook = None
    m = types.ModuleType("antenv.axon_hooks")
    m.get_axon_ntff_profile_hook = lambda: hook
    m.set_axon_ntff_profile_hook = lambda h: None
    sys.modules["antenv.axon_hooks"] = m


_install_ntff_shim()

from concourse import bass, mybir, tile  # noqa: E402
from concourse.bass_utils import run_bass_kernel_spmd  # noqa: E402
from concourse.vector_clock import ScopedClock  # noqa: E402

N_CORES = 8
F32 = mybir.dt.float32
BF16 = mybir.dt.bfloat16

# Device-side output dtype for the fast path. "bf16" halves the HBM write
# traffic (the dominant cost); host upcasts to float32. Output values are
# exp(-z^2/2) in (0, 1], so bf16 keeps max elementwise rel err <= 2^-8.
OUT_DTYPE = "bf16"


# ---------------------------------------------------------------------------
# Compat patches: this walrus build allows at most ONE sync-wait per
# instruction.  (1) split the Tile tail-drain waits over several drains;
# (2) post-pass that rewrites any multi-wait instruction into single-wait
# NoOps (same engine, immediately before) + the instruction with one wait.
# ---------------------------------------------------------------------------

# Tail style knob: "full" = drain + barrier + sem clear + barrier (safe);
# "slim" = drain + single barrier, no sem clears (the NEFF postamble zeroes
# every semaphore anyway — observed in traces).
TAIL_STYLE = "slim"

# Fast-path schedule: (n_units, issuing_engine) per group, covering all
# t_tiles*k units in order.  Small groups at the ends prime the DMA pipeline
# early and shrink the final drain; big groups amortize instruction overhead
# mid-stream.  'sync' and 'scalar' alternate so both HWDGE queues share the
# bytes (~300 B/ns per-queue cap; ~570 aggregate).
PLAN = [
    (2, "sync"), (2, "scalar"),
    (4, "sync"), (4, "scalar"), (4, "sync"), (4, "scalar"), (4, "scalar"),
    (2, "sync"), (2, "scalar"), (2, "sync"), (2, "sync"),
]


def _patched_drain_and_barrier(self, tick_clock, wait_clock):
    drain_inst = self.nc.sync.drain()
    wait_clock.add_sem_waits(
        drain_inst.ins, ScopedClock({None: tick_clock.global_clock})
    )
    si = drain_inst.ins.sync_info
    waits = list(si.on_wait) if si is not None else []
    if len(waits) > 1:
        drain_inst.ins.sync_info = mybir.SyncInfo(
            on_wait=waits[:1], on_update=list(si.on_update)
        )
        for w in waits[1:]:
            extra = self.nc.sync.drain()
            extra.ins.sync_info = mybir.SyncInfo(on_wait=[w], on_update=[])
    assert self.sems is not None
    popped = self.nc._tile_sem_poison_stack.pop()
    assert popped is self._sem_poison
    if TAIL_STYLE == "full":
        self.nc.all_engine_barrier()
        self.nc.clear_and_free_semaphores(list(self.sems.allocated().values()))
        self.nc.all_engine_barrier()
    else:
        self.nc.all_engine_barrier()


tile.TileContext._drain_and_barrier = _patched_drain_and_barrier


def _split_multi_waits(nc):
    """Rewrite every instruction carrying >1 sem wait: extra waits move onto
    fresh single-wait NoOps on the same engine placed immediately before."""
    eng_by_type = {
        mybir.EngineType.SP: nc.sync,
        mybir.EngineType.Activation: nc.scalar,
        mybir.EngineType.DVE: nc.vector,
        mybir.EngineType.Pool: nc.gpsimd,
        mybir.EngineType.PE: nc.tensor,
    }
    for fn in nc.m.functions:
        for bb in fn.blocks:
            insts = list(bb.instructions)
            if not any(
                i.sync_info is not None and len(i.sync_info.on_wait) > 1
                for i in insts
            ):
                continue
            new_list = []
            for inst in insts:
                si = inst.sync_info
                if si is not None and len(si.on_wait) > 1:
                    waits = list(si.on_wait)
                    for w in waits[:-1]:
                        nop = eng_by_type[inst.engine].nop(nofuse=True)
                        # nop() appended itself to the current bb; steal it.
                        cur = nc.cur_bb.bb
                        assert cur.instructions[-1] is nop.ins
                        cur.instructions.pop()
                        nop.ins.sync_info = mybir.SyncInfo(
                            on_wait=[w], on_update=[]
                        )
                        new_list.append(nop.ins)
                    inst.sync_info = mybir.SyncInfo(
                        on_wait=[waits[-1]], on_update=list(si.on_update)
                    )
                new_list.append(inst)
            bb.instructions[:] = new_list


# ---------------------------------------------------------------------------
# Device kernel builder
# ---------------------------------------------------------------------------

def _build_nc_fast(in_f, out_f, b_core, plan):
    """Fast path: out = Exp(alpha * x^2 + lnc), alpha/lnc per-partition.

    Per plan group of n units:
      DVE: y[:, u] = x2 * alpha[col] + lnc[col]   (tensor_scalar, 2x fp32)
      ACT: g = Exp(y)                             (one grouped instruction)
      DMA: g -> out pair-blocks                   (SP or ACT HWDGE queue)

    DRAM out layout: pair-blocks [n_units_total//2, 128, 2*b_core]; global
    unit u = t*k + r lives in block u//2, half u%2.
    """
    k = out_f // in_f
    t_tiles = in_f // 128
    ncols = t_tiles * k
    assert sum(n for n, _ in plan) == ncols
    assert all(n % 2 == 0 for n, _ in plan)

    odt = BF16 if OUT_DTYPE == "bf16" else F32
    nc = bass.Bass()
    xt_h = nc.declare_dram_parameter("xt", [t_tiles, 128, b_core], F32, isOutput=False)
    # p0 (alpha) and plc (lnc) merged: [128, 2*ncols], alpha cols then lnc cols
    pp_h = nc.declare_dram_parameter("pp", [128, 2 * ncols], F32, isOutput=False)
    out_h = nc.declare_dram_parameter(
        "out", [ncols // 2, 128, 2 * b_core], odt, isOutput=True
    )

    with tile.TileContext(nc) as tc:
        with (
            tc.tile_pool(name="params", bufs=1) as ppool,
            tc.tile_pool(name="xin", bufs=2) as xpool,
            tc.tile_pool(name="xsq", bufs=2) as x2pool,
            tc.tile_pool(name="ybuf", bufs=5) as ypool,
            tc.tile_pool(name="gbuf", bufs=5) as gpool,
        ):
            # params ride the GpSimd SWDGE queue: lands early, off the
            # critical SP queue
            pp = ppool.tile([128, 2 * ncols], F32)
            nc.gpsimd.dma_start(out=pp[:], in_=pp_h[:])

            xs = []
            # x0 in piece-DMAs on SP (whose queue is idle until the first
            # group is produced anyway); later x tiles ride the GpSimd
            # SWDGE queue (plenty of lead time)
            qtr = b_core // 4
            h = b_core // 2
            x0 = xpool.tile([128, b_core], F32, tag="x0")
            nc.sync.dma_start(out=x0[:, :qtr], in_=xt_h[0][:, :qtr])
            nc.sync.dma_start(out=x0[:, qtr:h], in_=xt_h[0][:, qtr:h])
            nc.sync.dma_start(out=x0[:, h:], in_=xt_h[0][:, h:])
            xs.append(x0)
            for t in range(1, t_tiles):
                xt = xpool.tile([128, b_core], F32, tag=f"x{t}")
                nc.gpsimd.dma_start(out=xt[:], in_=xt_h[t])
                xs.append(xt)

            def alpha(col):
                return pp[:, col : col + 1]

            def lnc(col):
                return pp[:, ncols + col : ncols + col + 1]

            x2s = {}

            def get_x2(t):
                if t not in x2s:
                    x2 = x2pool.tile([128, b_core], F32, tag=f"x2_{t}")
                    nc.vector.tensor_tensor(
                        x2[:], xs[t][:], xs[t][:], mybir.AluOpType.mult
                    )
                    x2s[t] = x2
                return x2s[t]

            def emit_group(u0, nu, eng_name, bsl=None):
                """Emit DVE ts + ACT exp + DMA for units [u0, u0+nu).
                bsl: optional (b_lo, b_hi) batch sub-range (priming)."""
                b_lo, b_hi = bsl if bsl else (0, b_core)
                bn = b_hi - b_lo
                key = (u0, nu)
                if key in gtiles:
                    g, y = gtiles[key]
                else:
                    g = gpool.tile([128, nu * b_core], odt, tag="g")
                    y = ypool.tile([128, nu * b_core], F32, tag="y")
                    gtiles[key] = (g, y)
                for j in range(nu):
                    col = u0 + j
                    x2 = get_x2(col // k)
                    sl = slice(j * b_core + b_lo, j * b_core + b_hi)
                    nc.vector.tensor_scalar(
                        y[:, sl], x2[:, b_lo:b_hi], alpha(col), lnc(col),
                        mybir.AluOpType.mult, mybir.AluOpType.add,
                    )
                if bsl is None:
                    y_ap, g_ap = y[:], g[:]
                    dram_ap = out_h[u0 // 2 : (u0 + nu) // 2].rearrange(
                        "c p f -> p c f"
                    )
                else:
                    y_ap = y[:].rearrange("p (j b) -> p j b", j=nu)[
                        :, :, b_lo:b_hi
                    ]
                    g_ap = g[:].rearrange("p (j b) -> p j b", j=nu)[
                        :, :, b_lo:b_hi
                    ]
                    dram_ap = out_h[u0 // 2 : (u0 + nu) // 2].rearrange(
                        "c p (j b) -> p (c j) b", j=2
                    )[:, :, b_lo:b_hi]
                nc.scalar.activation(
                    g_ap, y_ap, mybir.ActivationFunctionType.Exp
                )
                getattr(nc, eng_name).dma_start(out=dram_ap, in_=g_ap)

            gtiles = {}
            start_gi, u0 = 0, 0
            prime = len(plan) >= 2 and plan[0][0] == 2 and plan[1][0] == 2
            if prime:
                # prime the pipeline: first two groups computed and shipped
                # in batch quarters/halves, starting on x0's first piece
                x2_0 = x2pool.tile([128, b_core], F32, tag="x2_0")
                x2s[0] = x2_0
                for lo, hi in ((0, qtr), (qtr, h), (h, b_core)):
                    nc.vector.tensor_tensor(
                        x2_0[:, lo:hi], x0[:, lo:hi], x0[:, lo:hi],
                        mybir.AluOpType.mult,
                    )
                    emit_group(0, 2, plan[0][1], bsl=(lo, hi))
                    if lo == qtr:
                        emit_group(2, 2, plan[1][1], bsl=(0, h))
                emit_group(2, 2, plan[1][1], bsl=(h, b_core))
                if t_tiles > 1:
                    get_x2(1)
                start_gi, u0 = 2, 4
            for gi in range(start_gi, len(plan)):
                nu, eng_name = plan[gi]
                emit_group(u0, nu, eng_name)
                if gi == 0 and t_tiles > 1:
                    get_x2(1)
                u0 += nu

    _split_multi_waits(nc)
    return nc


def _build_nc(in_f, out_f, b_core, mode, group_u=4):
    """mode: 'general' -> out = Exp(-Square(a*x + b) + lnc) params: p0, p1, plc
             'cmul'    -> general + extra per-partition multiply by p2
    """
    k = out_f // in_f          # out features per input feature
    t_tiles = in_f // 128      # input-feature tiles
    n_groups = k // group_u
    ncols = t_tiles * k

    nc = bass.Bass()
    xt_h = nc.declare_dram_parameter("xt", [t_tiles, 128, b_core], F32, isOutput=False)
    p0_h = nc.declare_dram_parameter("p0", [128, ncols], F32, isOutput=False)
    plc_h = nc.declare_dram_parameter("plc", [128, ncols], F32, isOutput=False)
    if mode in ("general", "cmul"):
        p1_h = nc.declare_dram_parameter("p1", [128, ncols], F32, isOutput=False)
    if mode == "cmul":
        p2_h = nc.declare_dram_parameter("p2", [128, ncols], F32, isOutput=False)
    out_h = nc.declare_dram_parameter(
        "out", [t_tiles, n_groups, 128, group_u * b_core], F32, isOutput=True
    )

    with tile.TileContext(nc) as tc:
        with (
            tc.tile_pool(name="params", bufs=1) as ppool,
            tc.tile_pool(name="xin", bufs=2) as xpool,
            tc.tile_pool(name="xsq", bufs=2) as x2pool,
            tc.tile_pool(name="ybuf", bufs=3) as ypool,
            tc.tile_pool(name="gbuf", bufs=4) as gpool,
        ):
            # x tiles first: they head the critical chain to the first
            # output DMA; params load in parallel on other queues.
            xs = []
            for t in range(t_tiles):
                x = xpool.tile([128, b_core], F32, tag=f"x{t}")
                nc.sync.dma_start(out=x[:], in_=xt_h[t])
                xs.append(x)
            p0 = ppool.tile([128, ncols], F32)
            nc.sync.dma_start(out=p0[:], in_=p0_h[:])
            plc = ppool.tile([128, ncols], F32)
            nc.sync.dma_start(out=plc[:], in_=plc_h[:])
            if mode in ("general", "cmul"):
                p1 = ppool.tile([128, ncols], F32)
                nc.sync.dma_start(out=p1[:], in_=p1_h[:])
            if mode == "cmul":
                p2 = ppool.tile([128, ncols], F32)
                nc.sync.dma_start(out=p2[:], in_=p2_h[:])

            for t in range(t_tiles):
                x = xs[t]
                for q in range(n_groups):
                    g = gpool.tile([128, group_u * b_core], F32)
                    y = ypool.tile([128, group_u * b_core], F32)
                    for u in range(group_u):
                        col = t * k + q * group_u + u
                        sl = slice(u * b_core, (u + 1) * b_core)
                        # s = (a*x + b)^2  (per-partition a, b)
                        nc.scalar.activation(
                            y[:, sl],
                            x[:],
                            mybir.ActivationFunctionType.Square,
                            bias=p1[:, col : col + 1],
                            scale=p0[:, col : col + 1],
                        )
                        # g = exp(-s + lnc)
                        nc.scalar.activation(
                            g[:, sl],
                            y[:, sl],
                            mybir.ActivationFunctionType.Exp,
                            bias=plc[:, col : col + 1],
                            scale=-1.0,
                        )
                        if mode == "cmul":
                            nc.vector.tensor_scalar(
                                g[:, sl],
                                g[:, sl],
                                p2[:, col : col + 1],
                                None,
                                mybir.AluOpType.mult,
                            )
                    nc.sync.dma_start(out=out_h[t, q], in_=g[:])

    _split_multi_waits(nc)
    return nc


# ---------------------------------------------------------------------------
# Host orchestration
# ---------------------------------------------------------------------------

def _prep(x, expansion_mapping, means, vars_, coefs):
    x = np.asarray(x, dtype=np.float32)
    em = np.asarray(expansion_mapping, dtype=np.float32)
    means = np.asarray(means, dtype=np.float32)
    vars_ = np.asarray(vars_, dtype=np.float32)
    coefs = np.asarray(coefs, dtype=np.float32)
    return x, em, means, vars_, coefs


def _numpy_fallback(x, em, means, vars_, coefs):
    xe = x @ em.T
    z = (xe - means) / vars_
    return (np.exp(-0.5 * z * z) * coefs).astype(np.float32)


def _run(inputs, trace=False):
    x, em, means, vars_, coefs = _prep(
        inputs["x"], inputs["expansion_mapping"], inputs["means"],
        inputs["vars_"], inputs["coefs"],
    )
    batch, in_f = x.shape
    out_f = em.shape[0]

    idx = em.argmax(axis=1)
    scale_em = em[np.arange(out_f), idx]
    counts = np.bincount(idx, minlength=in_f)
    k = out_f // in_f
    structured = (
        batch % N_CORES == 0
        and in_f % 128 == 0
        and out_f % in_f == 0
        and np.all(counts == k)
        and k % 2 == 0
        # each row must be one-hot (a single nonzero)
        and np.count_nonzero(em) == out_f
    )
    if not structured:
        return _numpy_fallback(x, em, means, vars_, coefs), None

    b_core = batch // N_CORES
    t_tiles = in_f // 128
    ncols = t_tiles * k

    order = np.argsort(idx, kind="stable")  # device row m -> out feature
    a = scale_em[order] / vars_[order]
    b = -means[order] / vars_[order]
    c = coefs[order]

    fast = bool(np.all(b == 0.0))
    if np.all(c >= 0.0):
        with np.errstate(divide="ignore"):
            lc = np.log(c).astype(np.float32)
        cmul = None
    else:
        lc = np.zeros(out_f, dtype=np.float32)
        cmul = c

    if fast and cmul is None:
        mode = "fast"
        p0 = (-0.5 * a * a).astype(np.float32)
    elif cmul is None:
        mode = "general"
        p0 = (a / np.sqrt(2.0)).astype(np.float32)
        p1 = (b / np.sqrt(2.0)).astype(np.float32)
    else:
        mode = "cmul"
        p0 = (a / np.sqrt(2.0)).astype(np.float32)
        p1 = (b / np.sqrt(2.0)).astype(np.float32)

    def dev_layout(p):
        # full [out_f] ordered by device row m = 16*i + r -> [128, ncols]
        return np.ascontiguousarray(
            p.reshape(t_tiles, 128, k).transpose(1, 0, 2).reshape(128, ncols)
        )

    if mode == "fast":
        if ncols == sum(n for n, _ in PLAN):
            plan = PLAN
        else:
            plan = [
                (2, ("sync", "scalar")[i % 2]) for i in range(ncols // 2)
            ]
        nc = _build_nc_fast(in_f, out_f, b_core, plan)
        group_u = n_groups = None
    else:
        group_u = 4 if k % 4 == 0 else k
        n_groups = k // group_u
        nc = _build_nc(in_f, out_f, b_core, mode, group_u)

    xT = np.ascontiguousarray(x.T)  # [in_f, batch]
    if mode == "fast":
        base = {
            "pp": np.ascontiguousarray(
                np.hstack([dev_layout(p0), dev_layout(lc)])
            )
        }
    else:
        base = {"p0": dev_layout(p0), "plc": dev_layout(lc)}
        base["p1"] = dev_layout(p1)
        if mode == "cmul":
            base["p2"] = dev_layout(cmul)
    in_maps = []
    for kcore in range(N_CORES):
        shard = np.ascontiguousarray(
            xT[:, kcore * b_core : (kcore + 1) * b_core]
        ).reshape(t_tiles, 128, b_core)
        in_maps.append({"xt": shard, **base})

    res = run_bass_kernel_spmd(
        nc, in_maps, core_ids=list(range(N_CORES)), trace=trace
    )

    identity_order = bool(np.array_equal(order, np.arange(out_f)))
    out = np.empty((batch, out_f), dtype=np.float32)
    for kcore in range(N_CORES):
        dev = res.results[kcore]["out"]
        if mode == "fast":
            # [ncols//2, 128, 2*b_core]: block = global unit pair
            if dev.dtype != np.float32:
                dev = dev.astype(np.float32)
            dev = dev.reshape(t_tiles, k // 2, 128, 2, b_core)
        else:
            # [t, q, 128, group_u*b_core]
            dev = dev.reshape(t_tiles, n_groups, 128, group_u, b_core)
        # row m = 16*(128*t + p) + r  with r = (pairing axes 1,3)
        dev = dev.transpose(0, 2, 1, 3, 4).reshape(out_f, b_core)
        sl = slice(kcore * b_core, (kcore + 1) * b_core)
        if identity_order:
            out[sl] = dev.T
        else:
            out[sl.start : sl.stop, order] = dev.T
    return out, res


def kernel(**inputs):
    out, _ = _run(inputs, trace=False)
    return out


# revision 42
# speedup vs baseline: 1.3147x; 1.0192x over previous
"""Trainium2 Bass kernel for AdaptiveURBFLayer.

Computation:  out[b, j] = coefs[j] * exp(-0.5 * ((xe[b,j]-means[j])/vars_[j])^2)
where xe = x @ expansion_mapping.T and expansion_mapping is a (scaled) one-hot
row matrix: out feature j reads exactly one input feature idx[j].

Strategy (8 NeuronCores, data-parallel over batch):
  - Host extracts idx[j] = argmax(em[j]) and the scale em[j, idx[j]]; out
    features are grouped by source input feature (16 per feature here).
  - Device layout puts input features on SBUF partitions: x^T tiles
    [128, B_CORE].  Every group-of-16 output feature then has its RBF params
    as per-partition scalars.
  - Fast path (means folded to zero bias): out = exp(alpha*x^2 + lnc) with
    alpha = -0.5*(scale/vars)^2 per partition:
        DVE:  x2 = x*x (once per input tile)
              y  = x2*alpha + lnc   (tensor_scalar, 2x fp32 mode)
        ACT:  g  = Exp(y)           (grouped over 4 output units)
        DMA:  g -> DRAM (contiguous 2MB blocks)
  - Output is written feature-major per core; host transposes back.
"""

import sys
import types

import numpy as np


def _install_ntff_shim():
    """Make antenv.axon_hooks importable so run_bass_kernel_spmd(trace=True)
    can capture NTFF profiles under axon. Harmless if unavailable."""
    try:
        from antenv.axon_hooks import get_axon_ntff_profile_hook  # noqa: F401
        return
    except ImportError:
        pass
    try:
        from trn_agent_boot.trn_boot import _ntff_profile_via_ctypes
        hook = _ntff_profile_via_ctypes("/opt/axon/libaxon_pjrt.so")
    except Exception:
        hook = None
    m = types.ModuleType("antenv.axon_hooks")
    m.get_axon_ntff_profile_hook = lambda: hook
    m.set_axon_ntff_profile_hook = lambda h: None
    sys.modules["antenv.axon_hooks"] = m


_install_ntff_shim()

from concourse import bass, mybir, tile  # noqa: E402
from concourse.bass_utils import run_bass_kernel_spmd  # noqa: E402
from concourse.vector_clock import ScopedClock  # noqa: E402

N_CORES = 8
F32 = mybir.dt.float32
BF16 = mybir.dt.bfloat16

# Device-side output dtype for the fast path. "bf16" halves the HBM write
# traffic (the dominant cost); host upcasts to float32. Output values are
# exp(-z^2/2) in (0, 1], so bf16 keeps max elementwise rel err <= 2^-8.
OUT_DTYPE = "bf16"


# ---------------------------------------------------------------------------
# Compat patches: this walrus build allows at most ONE sync-wait per
# instruction.  (1) split the Tile tail-drain waits over several drains;
# (2) post-pass that rewrites any multi-wait instruction into single-wait
# NoOps (same engine, immediately before) + the instruction with one wait.
# ---------------------------------------------------------------------------

# Tail style knob: "full" = drain + barrier + sem clear + barrier (safe);
# "slim" = drain + single barrier, no sem clears (the NEFF postamble zeroes
# every semaphore anyway — observed in traces).
TAIL_STYLE = "slim"

# Fast-path schedule: (n_units, issuing_engine) per group, covering all
# t_tiles*k units in order.  Small groups at the ends prime the DMA pipeline
# early and shrink the final drain; big groups amortize instruction overhead
# mid-stream.  'sync' and 'scalar' alternate so both HWDGE queues share the
# bytes (~300 B/ns per-queue cap; ~570 aggregate).
PLAN = [
    (2, "sync"), (2, "sync"),
    (4, "sync"), (4, "sync"), (4, "sync"), (4, "sync"),
    (4, "sync"), (4, "sync"), (4, "sync"),
]

# prime the first groups through batch-sliced sub-pipelines (helps only when
# the output DMA is the wall; pads the ACT stream otherwise)
PRIME = False


def _patched_drain_and_barrier(self, tick_clock, wait_clock):
    drain_inst = self.nc.sync.drain()
    wait_clock.add_sem_waits(
        drain_inst.ins, ScopedClock({None: tick_clock.global_clock})
    )
    si = drain_inst.ins.sync_info
    waits = list(si.on_wait) if si is not None else []
    if len(waits) > 1:
        drain_inst.ins.sync_info = mybir.SyncInfo(
            on_wait=waits[:1], on_update=list(si.on_update)
        )
        for w in waits[1:]:
            extra = self.nc.sync.drain()
            extra.ins.sync_info = mybir.SyncInfo(on_wait=[w], on_update=[])
    assert self.sems is not None
    popped = self.nc._tile_sem_poison_stack.pop()
    assert popped is self._sem_poison
    if TAIL_STYLE == "full":
        self.nc.all_engine_barrier()
        self.nc.clear_and_free_semaphores(list(self.sems.allocated().values()))
        self.nc.all_engine_barrier()
    else:
        self.nc.all_engine_barrier()


tile.TileContext._drain_and_barrier = _patched_drain_and_barrier


def _split_multi_waits(nc):
    """Rewrite every instruction carrying >1 sem wait: extra waits move onto
    fresh single-wait NoOps on the same engine placed immediately before."""
    eng_by_type = {
        mybir.EngineType.SP: nc.sync,
        mybir.EngineType.Activation: nc.scalar,
        mybir.EngineType.DVE: nc.vector,
        mybir.EngineType.Pool: nc.gpsimd,
        mybir.EngineType.PE: nc.tensor,
    }
    for fn in nc.m.functions:
        for bb in fn.blocks:
            insts = list(bb.instructions)
            if not any(
                i.sync_info is not None and len(i.sync_info.on_wait) > 1
                for i in insts
            ):
                continue
            new_list = []
            for inst in insts:
                si = inst.sync_info
                if si is not None and len(si.on_wait) > 1:
                    waits = list(si.on_wait)
                    for w in waits[:-1]:
                        nop = eng_by_type[inst.engine].nop(nofuse=True)
                        # nop() appended itself to the current bb; steal it.
                        cur = nc.cur_bb.bb
                        assert cur.instructions[-1] is nop.ins
                        cur.instructions.pop()
                        nop.ins.sync_info = mybir.SyncInfo(
                            on_wait=[w], on_update=[]
                        )
                        new_list.append(nop.ins)
                    inst.sync_info = mybir.SyncInfo(
                        on_wait=[waits[-1]], on_update=list(si.on_update)
                    )
                new_list.append(inst)
            bb.instructions[:] = new_list


# ---------------------------------------------------------------------------
# Device kernel builder
# ---------------------------------------------------------------------------

def _build_nc_fast(in_f, out_f, b_core, plan):
    """Fast path: out = Exp(alpha * x^2 + lnc), alpha/lnc per-partition.

    Per plan group of n units:
      DVE: y[:, u] = x2 * alpha[col] + lnc[col]   (tensor_scalar, 2x fp32)
      ACT: g = Exp(y)                             (one grouped instruction)
      DMA: g -> out pair-blocks                   (SP or ACT HWDGE queue)

    DRAM out layout: pair-blocks [n_units_total//2, 128, 2*b_core]; global
    unit u = t*k + r lives in block u//2, half u%2.
    """
    k = out_f // in_f
    t_tiles = in_f // 128
    ncols = t_tiles * k
    assert sum(n for n, _ in plan) == ncols
    assert all(n % 2 == 0 for n, _ in plan)

    odt = BF16 if OUT_DTYPE == "bf16" else F32
    nc = bass.Bass()
    xt_h = nc.declare_dram_parameter("xt", [t_tiles, 128, b_core], F32, isOutput=False)
    # p0 (alpha) and plc (lnc) merged: [128, 2*ncols], alpha cols then lnc cols
    pp_h = nc.declare_dram_parameter("pp", [128, 2 * ncols], F32, isOutput=False)
    out_h = nc.declare_dram_parameter(
        "out", [ncols // 2, 128, 2 * b_core], odt, isOutput=True
    )

    with tile.TileContext(nc) as tc:
        with (
            tc.tile_pool(name="params", bufs=1) as ppool,
            tc.tile_pool(name="xin", bufs=2) as xpool,
            tc.tile_pool(name="xsq", bufs=2) as x2pool,
            tc.tile_pool(name="ybuf", bufs=5) as ypool,
            tc.tile_pool(name="gbuf", bufs=5) as gpool,
        ):
            # params ride the GpSimd SWDGE queue: lands early, off the
            # critical SP queue
            pp = ppool.tile([128, 2 * ncols], F32)
            nc.gpsimd.dma_start(out=pp[:], in_=pp_h[:])

            xs = []
            # x0 on SP (whose queue is idle until the first group is
            # produced anyway); later x tiles ride the GpSimd SWDGE queue
            # (plenty of lead time)
            qtr = b_core // 4
            h = b_core // 2
            x0 = xpool.tile([128, b_core], F32, tag="x0")
            if PRIME:
                nc.sync.dma_start(out=x0[:, :qtr], in_=xt_h[0][:, :qtr])
                nc.sync.dma_start(out=x0[:, qtr:h], in_=xt_h[0][:, qtr:h])
                nc.sync.dma_start(out=x0[:, h:], in_=xt_h[0][:, h:])
            else:
                nc.sync.dma_start(out=x0[:], in_=xt_h[0])
            xs.append(x0)
            for t in range(1, t_tiles):
                xt = xpool.tile([128, b_core], F32, tag=f"x{t}")
                nc.gpsimd.dma_start(out=xt[:], in_=xt_h[t])
                xs.append(xt)

            def alpha(col):
                return pp[:, col : col + 1]

            def lnc(col):
                return pp[:, ncols + col : ncols + col + 1]

            x2s = {}

            def get_x2(t):
                if t not in x2s:
                    x2 = x2pool.tile([128, b_core], F32, tag=f"x2_{t}")
                    nc.vector.tensor_tensor(
                        x2[:], xs[t][:], xs[t][:], mybir.AluOpType.mult
                    )
                    x2s[t] = x2
                return x2s[t]

            def emit_group(u0, nu, eng_name, bsl=None):
                """Emit DVE ts + ACT exp + DMA for units [u0, u0+nu).
                bsl: optional (b_lo, b_hi) batch sub-range (priming)."""
                b_lo, b_hi = bsl if bsl else (0, b_core)
                bn = b_hi - b_lo
                key = (u0, nu)
                if key in gtiles:
                    g, y = gtiles[key]
                else:
                    g = gpool.tile([128, nu * b_core], odt, tag="g")
                    y = ypool.tile([128, nu * b_core], F32, tag="y")
                    gtiles[key] = (g, y)
                for j in range(nu):
                    col = u0 + j
                    x2 = get_x2(col // k)
                    sl = slice(j * b_core + b_lo, j * b_core + b_hi)
                    nc.vector.tensor_scalar(
                        y[:, sl], x2[:, b_lo:b_hi], alpha(col), lnc(col),
                        mybir.AluOpType.mult, mybir.AluOpType.add,
                    )
                if bsl is None:
                    y_ap, g_ap = y[:], g[:]
                    dram_ap = out_h[u0 // 2 : (u0 + nu) // 2].rearrange(
                        "c p f -> p c f"
                    )
                else:
                    y_ap = y[:].rearrange("p (j b) -> p j b", j=nu)[
                        :, :, b_lo:b_hi
                    ]
                    g_ap = g[:].rearrange("p (j b) -> p j b", j=nu)[
                        :, :, b_lo:b_hi
                    ]
                    dram_ap = out_h[u0 // 2 : (u0 + nu) // 2].rearrange(
                        "c p (j b) -> p (c j) b", j=2
                    )[:, :, b_lo:b_hi]
                nc.scalar.activation(
                    g_ap, y_ap, mybir.ActivationFunctionType.Exp
                )
                getattr(nc, eng_name).dma_start(out=dram_ap, in_=g_ap)

            gtiles = {}
            start_gi, u0 = 0, 0
            prime = (
                PRIME and len(plan) >= 2
                and plan[0][0] == 2 and plan[1][0] == 2
            )
            if prime:
                # prime the pipeline: first two groups computed and shipped
                # in batch quarters/halves, starting on x0's first piece
                x2_0 = x2pool.tile([128, b_core], F32, tag="x2_0")
                x2s[0] = x2_0
                for lo, hi in ((0, qtr), (qtr, h), (h, b_core)):
                    nc.vector.tensor_tensor(
                        x2_0[:, lo:hi], x0[:, lo:hi], x0[:, lo:hi],
                        mybir.AluOpType.mult,
                    )
                    emit_group(0, 2, plan[0][1], bsl=(lo, hi))
                    if lo == qtr:
                        emit_group(2, 2, plan[1][1], bsl=(0, h))
                emit_group(2, 2, plan[1][1], bsl=(h, b_core))
                if t_tiles > 1:
                    get_x2(1)
                start_gi, u0 = 2, 4
            for gi in range(start_gi, len(plan)):
                nu, eng_name = plan[gi]
                emit_group(u0, nu, eng_name)
                if gi == 0 and t_tiles > 1:
                    get_x2(1)
                u0 += nu

    _split_multi_waits(nc)
    return nc


def _build_nc(in_f, out_f, b_core, mode, group_u=4):
    """mode: 'general' -> out = Exp(-Square(a*x + b) + lnc) params: p0, p1, plc
             'cmul'    -> general + extra per-partition multiply by p2
    """
    k = out_f // in_f          # out features per input feature
    t_tiles = in_f // 128      # input-feature tiles
    n_groups = k // group_u
    ncols = t_tiles * k

    nc = bass.Bass()
    xt_h = nc.declare_dram_parameter("xt", [t_tiles, 128, b_core], F32, isOutput=False)
    p0_h = nc.declare_dram_parameter("p0", [128, ncols], F32, isOutput=False)
    plc_h = nc.declare_dram_parameter("plc", [128, ncols], F32, isOutput=False)
    if mode in ("general", "cmul"):
        p1_h = nc.declare_dram_parameter("p1", [128, ncols], F32, isOutput=False)
    if mode == "cmul":
        p2_h = nc.declare_dram_parameter("p2", [128, ncols], F32, isOutput=False)
    out_h = nc.declare_dram_parameter(
        "out", [t_tiles, n_groups, 128, group_u * b_core], F32, isOutput=True
    )

    with tile.TileContext(nc) as tc:
        with (
            tc.tile_pool(name="params", bufs=1) as ppool,
            tc.tile_pool(name="xin", bufs=2) as xpool,
            tc.tile_pool(name="xsq", bufs=2) as x2pool,
            tc.tile_pool(name="ybuf", bufs=3) as ypool,
            tc.tile_pool(name="gbuf", bufs=4) as gpool,
        ):
            # x tiles first: they head the critical chain to the first
            # output DMA; params load in parallel on other queues.
            xs = []
            for t in range(t_tiles):
                x = xpool.tile([128, b_core], F32, tag=f"x{t}")
                nc.sync.dma_start(out=x[:], in_=xt_h[t])
                xs.append(x)
            p0 = ppool.tile([128, ncols], F32)
            nc.sync.dma_start(out=p0[:], in_=p0_h[:])
            plc = ppool.tile([128, ncols], F32)
            nc.sync.dma_start(out=plc[:], in_=plc_h[:])
            if mode in ("general", "cmul"):
                p1 = ppool.tile([128, ncols], F32)
                nc.sync.dma_start(out=p1[:], in_=p1_h[:])
            if mode == "cmul":
                p2 = ppool.tile([128, ncols], F32)
                nc.sync.dma_start(out=p2[:], in_=p2_h[:])

            for t in range(t_tiles):
                x = xs[t]
                for q in range(n_groups):
                    g = gpool.tile([128, group_u * b_core], F32)
                    y = ypool.tile([128, group_u * b_core], F32)
                    for u in range(group_u):
                        col = t * k + q * group_u + u
                        sl = slice(u * b_core, (u + 1) * b_core)
                        # s = (a*x + b)^2  (per-partition a, b)
                        nc.scalar.activation(
                            y[:, sl],
                            x[:],
                            mybir.ActivationFunctionType.Square,
                            bias=p1[:, col : col + 1],
                            scale=p0[:, col : col + 1],
                        )
                        # g = exp(-s + lnc)
                        nc.scalar.activation(
                            g[:, sl],
                            y[:, sl],
                            mybir.ActivationFunctionType.Exp,
                            bias=plc[:, col : col + 1],
                            scale=-1.0,
                        )
                        if mode == "cmul":
                            nc.vector.tensor_scalar(
                                g[:, sl],
                                g[:, sl],
                                p2[:, col : col + 1],
                                None,
                                mybir.AluOpType.mult,
                            )
                    nc.sync.dma_start(out=out_h[t, q], in_=g[:])

    _split_multi_waits(nc)
    return nc


# ---------------------------------------------------------------------------
# Host orchestration
# ---------------------------------------------------------------------------

def _prep(x, expansion_mapping, means, vars_, coefs):
    x = np.asarray(x, dtype=np.float32)
    em = np.asarray(expansion_mapping, dtype=np.float32)
    means = np.asarray(means, dtype=np.float32)
    vars_ = np.asarray(vars_, dtype=np.float32)
    coefs = np.asarray(coefs, dtype=np.float32)
    return x, em, means, vars_, coefs


def _numpy_fallback(x, em, means, vars_, coefs):
    xe = x @ em.T
    z = (xe - means) / vars_
    return (np.exp(-0.5 * z * z) * coefs).astype(np.float32)


def _run(inputs, trace=False):
    x, em, means, vars_, coefs = _prep(
        inputs["x"], inputs["expansion_mapping"], inputs["means"],
        inputs["vars_"], inputs["coefs"],
    )
    batch, in_f = x.shape
    out_f = em.shape[0]

    idx = em.argmax(axis=1)
    scale_em = em[np.arange(out_f), idx]
    counts = np.bincount(idx, minlength=in_f)
    k = out_f // in_f
    structured = (
        batch % N_CORES == 0
        and in_f % 128 == 0
        and out_f % in_f == 0
        and np.all(counts == k)
        and k % 2 == 0
        # each row must be one-hot (a single nonzero)
        and np.count_nonzero(em) == out_f
    )
    if not structured:
        return _numpy_fallback(x, em, means, vars_, coefs), None

    b_core = batch // N_CORES
    t_tiles = in_f // 128
    ncols = t_tiles * k

    order = np.argsort(idx, kind="stable")  # device row m -> out feature
    a = scale_em[order] / vars_[order]
    b = -means[order] / vars_[order]
    c = coefs[order]

    fast = bool(np.all(b == 0.0))
    if np.all(c >= 0.0):
        with np.errstate(divide="ignore"):
            lc = np.log(c).astype(np.float32)
        cmul = None
    else:
        lc = np.zeros(out_f, dtype=np.float32)
        cmul = c

    if fast and cmul is None:
        mode = "fast"
        p0 = (-0.5 * a * a).astype(np.float32)
    elif cmul is None:
        mode = "general"
        p0 = (a / np.sqrt(2.0)).astype(np.float32)
        p1 = (b / np.sqrt(2.0)).astype(np.float32)
    else:
        mode = "cmul"
        p0 = (a / np.sqrt(2.0)).astype(np.float32)
        p1 = (b / np.sqrt(2.0)).astype(np.float32)

    def dev_layout(p):
        # full [out_f] ordered by device row m = 16*i + r -> [128, ncols]
        return np.ascontiguousarray(
            p.reshape(t_tiles, 128, k).transpose(1, 0, 2).reshape(128, ncols)
        )

    if mode == "fast":
        if ncols == sum(n for n, _ in PLAN):
            plan = PLAN
        else:
            plan = [
                (2, ("sync", "scalar")[i % 2]) for i in range(ncols // 2)
            ]
        nc = _build_nc_fast(in_f, out_f, b_core, plan)
        group_u = n_groups = None
    else:
        group_u = 4 if k % 4 == 0 else k
        n_groups = k // group_u
        nc = _build_nc(in_f, out_f, b_core, mode, group_u)

    xT = np.ascontiguousarray(x.T)  # [in_f, batch]
    if mode == "fast":
        base = {
            "pp": np.ascontiguousarray(
                np.hstack([dev_layout(p0), dev_layout(lc)])
            )
        }
    else:
        base = {"p0": dev_layout(p0), "plc": dev_layout(lc)}
        base["p1"] = dev_layout(p1)
        if mode == "cmul":
            base["p2"] = dev_layout(cmul)
    in_maps = []
    for kcore in range(N_CORES):
        shard = np.ascontiguousarray(
            xT[:, kcore * b_core : (kcore + 1) * b_core]
        ).reshape(t_tiles, 128, b_core)
        in_maps.append({"xt": shard, **base})

    res = run_bass_kernel_spmd(
        nc, in_maps, core_ids=list(range(N_CORES)), trace=trace
    )

    identity_order = bool(np.array_equal(order, np.arange(out_f)))
    out = np.empty((batch, out_f), dtype=np.float32)
    for kcore in range(N_CORES):
        dev = res.results[kcore]["out"]
        if mode == "fast":
            # [ncols//2, 128, 2*b_core]: block = global unit pair
            if dev.dtype != np.float32:
                dev = dev.astype(np.float32)
            dev = dev.reshape(t_tiles, k // 2, 128, 2, b_core)
        else:
            # [t, q, 128, group_u*b_core]
            dev = dev.reshape(t_tiles, n_groups, 128, group_u, b_core)
        # row m = 16*(128*t + p) + r  with r = (pairing axes 1,3)
        dev = dev.transpose(0, 2, 1, 3, 4).reshape(out_f, b_core)
        sl = slice(kcore * b_core, (kcore + 1) * b_core)
        if identity_order:
            out[sl] = dev.T
        else:
            out[sl.start : sl.stop, order] = dev.T
    return out, res


def kernel(**inputs):
    out, _ = _run(inputs, trace=False)
    return out


# revision 46
# speedup vs baseline: 1.3487x; 1.0258x over previous
"""Trainium2 Bass kernel for AdaptiveURBFLayer.

Computation:  out[b, j] = coefs[j] * exp(-0.5 * ((xe[b,j]-means[j])/vars_[j])^2)
where xe = x @ expansion_mapping.T and expansion_mapping is a (scaled) one-hot
row matrix: out feature j reads exactly one input feature idx[j].

Strategy (8 NeuronCores, data-parallel over batch):
  - Host extracts idx[j] = argmax(em[j]) and the scale em[j, idx[j]]; out
    features are grouped by source input feature (16 per feature here).
  - Device layout puts input features on SBUF partitions: x^T tiles
    [128, B_CORE].  Every group-of-16 output feature then has its RBF params
    as per-partition scalars.
  - Fast path (means folded to zero bias): out = exp(alpha*x^2 + lnc) with
    alpha = -0.5*(scale/vars)^2 per partition:
        DVE:  x2 = x*x (once per input tile)
              y  = x2*alpha + lnc   (tensor_scalar, 2x fp32 mode)
        ACT:  g  = Exp(y)           (grouped over 4 output units)
        DMA:  g -> DRAM (contiguous 2MB blocks)
  - Output is written feature-major per core; host transposes back.
"""

import sys
import types

import numpy as np


def _install_ntff_shim():
    """Make antenv.axon_hooks importable so run_bass_kernel_spmd(trace=True)
    can capture NTFF profiles under axon. Harmless if unavailable."""
    try:
        from antenv.axon_hooks import get_axon_ntff_profile_hook  # noqa: F401
        return
    except ImportError:
        pass
    try:
        from trn_agent_boot.trn_boot import _ntff_profile_via_ctypes
        hook = _ntff_profile_via_ctypes("/opt/axon/libaxon_pjrt.so")
    except Exception:
        hook = None
    m = types.ModuleType("antenv.axon_hooks")
    m.get_axon_ntff_profile_hook = lambda: hook
    m.set_axon_ntff_profile_hook = lambda h: None
    sys.modules["antenv.axon_hooks"] = m


_install_ntff_shim()

from concourse import bass, mybir, tile  # noqa: E402
from concourse.bass_utils import run_bass_kernel_spmd  # noqa: E402
from concourse.vector_clock import ScopedClock  # noqa: E402

N_CORES = 8
F32 = mybir.dt.float32
BF16 = mybir.dt.bfloat16

# Device-side output dtype for the fast path. "bf16" halves the HBM write
# traffic (the dominant cost); host upcasts to float32. Output values are
# exp(-z^2/2) in (0, 1], so bf16 keeps max elementwise rel err <= 2^-8.
OUT_DTYPE = "bf16"


# ---------------------------------------------------------------------------
# Compat patches: this walrus build allows at most ONE sync-wait per
# instruction.  (1) split the Tile tail-drain waits over several drains;
# (2) post-pass that rewrites any multi-wait instruction into single-wait
# NoOps (same engine, immediately before) + the instruction with one wait.
# ---------------------------------------------------------------------------

# Tail style knob: "full" = drain + barrier + sem clear + barrier (safe);
# "slim" = drain + single barrier, no sem clears (the NEFF postamble zeroes
# every semaphore anyway — observed in traces).
TAIL_STYLE = "slim"

# Fast-path schedule: (n_units, issuing_engine) per group, covering all
# t_tiles*k units in order.  Small groups at the ends prime the DMA pipeline
# early and shrink the final drain; big groups amortize instruction overhead
# mid-stream.  'sync' and 'scalar' alternate so both HWDGE queues share the
# bytes (~300 B/ns per-queue cap; ~570 aggregate).
PLAN = [
    (2, "sync"), (2, "sync"),
    (4, "sync"), (4, "sync"), (4, "sync"), (4, "sync"),
    (4, "sync"), (4, "sync"),
    (2, "sync"), (2, "sync"),
]

# prime group 0 through half-batch sub-pipelines so the ACT exp stream (the
# end-to-end bottleneck) starts ~3us earlier
PRIME = True


def _patched_drain_and_barrier(self, tick_clock, wait_clock):
    drain_inst = self.nc.sync.drain()
    wait_clock.add_sem_waits(
        drain_inst.ins, ScopedClock({None: tick_clock.global_clock})
    )
    si = drain_inst.ins.sync_info
    waits = list(si.on_wait) if si is not None else []
    if len(waits) > 1:
        drain_inst.ins.sync_info = mybir.SyncInfo(
            on_wait=waits[:1], on_update=list(si.on_update)
        )
        for w in waits[1:]:
            extra = self.nc.sync.drain()
            extra.ins.sync_info = mybir.SyncInfo(on_wait=[w], on_update=[])
    assert self.sems is not None
    popped = self.nc._tile_sem_poison_stack.pop()
    assert popped is self._sem_poison
    if TAIL_STYLE == "full":
        self.nc.all_engine_barrier()
        self.nc.clear_and_free_semaphores(list(self.sems.allocated().values()))
        self.nc.all_engine_barrier()
    else:
        self.nc.all_engine_barrier()


tile.TileContext._drain_and_barrier = _patched_drain_and_barrier


def _split_multi_waits(nc):
    """Rewrite every instruction carrying >1 sem wait: extra waits move onto
    fresh single-wait NoOps on the same engine placed immediately before."""
    eng_by_type = {
        mybir.EngineType.SP: nc.sync,
        mybir.EngineType.Activation: nc.scalar,
        mybir.EngineType.DVE: nc.vector,
        mybir.EngineType.Pool: nc.gpsimd,
        mybir.EngineType.PE: nc.tensor,
    }
    for fn in nc.m.functions:
        for bb in fn.blocks:
            insts = list(bb.instructions)
            if not any(
                i.sync_info is not None and len(i.sync_info.on_wait) > 1
                for i in insts
            ):
                continue
            new_list = []
            for inst in insts:
                si = inst.sync_info
                if si is not None and len(si.on_wait) > 1:
                    waits = list(si.on_wait)
                    for w in waits[:-1]:
                        nop = eng_by_type[inst.engine].nop(nofuse=True)
                        # nop() appended itself to the current bb; steal it.
                        cur = nc.cur_bb.bb
                        assert cur.instructions[-1] is nop.ins
                        cur.instructions.pop()
                        nop.ins.sync_info = mybir.SyncInfo(
                            on_wait=[w], on_update=[]
                        )
                        new_list.append(nop.ins)
                    inst.sync_info = mybir.SyncInfo(
                        on_wait=[waits[-1]], on_update=list(si.on_update)
                    )
                new_list.append(inst)
            bb.instructions[:] = new_list


# ---------------------------------------------------------------------------
# Device kernel builder
# ---------------------------------------------------------------------------

def _build_nc_fast(in_f, out_f, b_core, plan):
    """Fast path: out = Exp(alpha * x^2 + lnc), alpha/lnc per-partition.

    Per plan group of n units:
      DVE: y[:, u] = x2 * alpha[col] + lnc[col]   (tensor_scalar, 2x fp32)
      ACT: g = Exp(y)                             (one grouped instruction)
      DMA: g -> out pair-blocks                   (SP or ACT HWDGE queue)

    DRAM out layout: pair-blocks [n_units_total//2, 128, 2*b_core]; global
    unit u = t*k + r lives in block u//2, half u%2.
    """
    k = out_f // in_f
    t_tiles = in_f // 128
    ncols = t_tiles * k
    assert sum(n for n, _ in plan) == ncols
    assert all(n % 2 == 0 for n, _ in plan)

    odt = BF16 if OUT_DTYPE == "bf16" else F32
    nc = bass.Bass()
    xt_h = nc.declare_dram_parameter("xt", [t_tiles, 128, b_core], F32, isOutput=False)
    # p0 (alpha) and plc (lnc) merged: [128, 2*ncols], alpha cols then lnc cols
    pp_h = nc.declare_dram_parameter("pp", [128, 2 * ncols], F32, isOutput=False)
    out_h = nc.declare_dram_parameter(
        "out", [ncols // 2, 128, 2 * b_core], odt, isOutput=True
    )

    with tile.TileContext(nc) as tc:
        with (
            tc.tile_pool(name="params", bufs=1) as ppool,
            tc.tile_pool(name="xin", bufs=2) as xpool,
            tc.tile_pool(name="xsq", bufs=2) as x2pool,
            tc.tile_pool(name="ybuf", bufs=5) as ypool,
            tc.tile_pool(name="gbuf", bufs=5) as gpool,
        ):
            # params ride the GpSimd SWDGE queue: lands early, off the
            # critical SP queue
            pp = ppool.tile([128, 2 * ncols], F32)
            nc.gpsimd.dma_start(out=pp[:], in_=pp_h[:])

            xs = []
            # x0 on SP (whose queue is idle until the first group is
            # produced anyway); later x tiles ride the GpSimd SWDGE queue
            # (plenty of lead time)
            qtr = b_core // 4
            h = b_core // 2
            x0 = xpool.tile([128, b_core], F32, tag="x0")
            if PRIME:
                nc.sync.dma_start(out=x0[:, :h], in_=xt_h[0][:, :h])
                nc.sync.dma_start(out=x0[:, h:], in_=xt_h[0][:, h:])
            else:
                nc.sync.dma_start(out=x0[:], in_=xt_h[0])
            xs.append(x0)
            for t in range(1, t_tiles):
                xt = xpool.tile([128, b_core], F32, tag=f"x{t}")
                nc.gpsimd.dma_start(out=xt[:], in_=xt_h[t])
                xs.append(xt)

            def alpha(col):
                return pp[:, col : col + 1]

            def lnc(col):
                return pp[:, ncols + col : ncols + col + 1]

            x2s = {}

            def get_x2(t):
                if t not in x2s:
                    x2 = x2pool.tile([128, b_core], F32, tag=f"x2_{t}")
                    nc.vector.tensor_tensor(
                        x2[:], xs[t][:], xs[t][:], mybir.AluOpType.mult
                    )
                    x2s[t] = x2
                return x2s[t]

            def emit_group(u0, nu, eng_name, bsl=None):
                """Emit DVE ts + ACT exp + DMA for units [u0, u0+nu).
                bsl: optional (b_lo, b_hi) batch sub-range (priming)."""
                b_lo, b_hi = bsl if bsl else (0, b_core)
                bn = b_hi - b_lo
                key = (u0, nu)
                if key in gtiles:
                    g, y = gtiles[key]
                else:
                    g = gpool.tile([128, nu * b_core], odt, tag="g")
                    y = ypool.tile([128, nu * b_core], F32, tag="y")
                    gtiles[key] = (g, y)
                for j in range(nu):
                    col = u0 + j
                    x2 = get_x2(col // k)
                    sl = slice(j * b_core + b_lo, j * b_core + b_hi)
                    nc.vector.tensor_scalar(
                        y[:, sl], x2[:, b_lo:b_hi], alpha(col), lnc(col),
                        mybir.AluOpType.mult, mybir.AluOpType.add,
                    )
                if bsl is None:
                    y_ap, g_ap = y[:], g[:]
                    dram_ap = out_h[u0 // 2 : (u0 + nu) // 2].rearrange(
                        "c p f -> p c f"
                    )
                else:
                    y_ap = y[:].rearrange("p (j b) -> p j b", j=nu)[
                        :, :, b_lo:b_hi
                    ]
                    g_ap = g[:].rearrange("p (j b) -> p j b", j=nu)[
                        :, :, b_lo:b_hi
                    ]
                    dram_ap = out_h[u0 // 2 : (u0 + nu) // 2].rearrange(
                        "c p (j b) -> p (c j) b", j=2
                    )[:, :, b_lo:b_hi]
                nc.scalar.activation(
                    g_ap, y_ap, mybir.ActivationFunctionType.Exp
                )
                getattr(nc, eng_name).dma_start(out=dram_ap, in_=g_ap)

            gtiles = {}
            start_gi, u0 = 0, 0
            prime = PRIME and len(plan) >= 1 and plan[0][0] == 2
            if prime:
                # prime the pipeline: group 0 computed and shipped in batch
                # halves, starting as soon as x0's first half lands
                x2_0 = x2pool.tile([128, b_core], F32, tag="x2_0")
                x2s[0] = x2_0
                for lo, hi in ((0, h), (h, b_core)):
                    nc.vector.tensor_tensor(
                        x2_0[:, lo:hi], x0[:, lo:hi], x0[:, lo:hi],
                        mybir.AluOpType.mult,
                    )
                    emit_group(0, 2, plan[0][1], bsl=(lo, hi))
                start_gi, u0 = 1, 2
            for gi in range(start_gi, len(plan)):
                nu, eng_name = plan[gi]
                emit_group(u0, nu, eng_name)
                # slot t=1's x^2 in after the first full group's ts ops so
                # it does not delay the pipeline head
                if gi == start_gi and t_tiles > 1:
                    get_x2(1)
                u0 += nu

    _split_multi_waits(nc)
    return nc


def _build_nc(in_f, out_f, b_core, mode, group_u=4):
    """mode: 'general' -> out = Exp(-Square(a*x + b) + lnc) params: p0, p1, plc
             'cmul'    -> general + extra per-partition multiply by p2
    """
    k = out_f // in_f          # out features per input feature
    t_tiles = in_f // 128      # input-feature tiles
    n_groups = k // group_u
    ncols = t_tiles * k

    nc = bass.Bass()
    xt_h = nc.declare_dram_parameter("xt", [t_tiles, 128, b_core], F32, isOutput=False)
    p0_h = nc.declare_dram_parameter("p0", [128, ncols], F32, isOutput=False)
    plc_h = nc.declare_dram_parameter("plc", [128, ncols], F32, isOutput=False)
    if mode in ("general", "cmul"):
        p1_h = nc.declare_dram_parameter("p1", [128, ncols], F32, isOutput=False)
    if mode == "cmul":
        p2_h = nc.declare_dram_parameter("p2", [128, ncols], F32, isOutput=False)
    out_h = nc.declare_dram_parameter(
        "out", [t_tiles, n_groups, 128, group_u * b_core], F32, isOutput=True
    )

    with tile.TileContext(nc) as tc:
        with (
            tc.tile_pool(name="params", bufs=1) as ppool,
            tc.tile_pool(name="xin", bufs=2) as xpool,
            tc.tile_pool(name="xsq", bufs=2) as x2pool,
            tc.tile_pool(name="ybuf", bufs=3) as ypool,
            tc.tile_pool(name="gbuf", bufs=4) as gpool,
        ):
            # x tiles first: they head the critical chain to the first
            # output DMA; params load in parallel on other queues.
            xs = []
            for t in range(t_tiles):
                x = xpool.tile([128, b_core], F32, tag=f"x{t}")
                nc.sync.dma_start(out=x[:], in_=xt_h[t])
                xs.append(x)
            p0 = ppool.tile([128, ncols], F32)
            nc.sync.dma_start(out=p0[:], in_=p0_h[:])
            plc = ppool.tile([128, ncols], F32)
            nc.sync.dma_start(out=plc[:], in_=plc_h[:])
            if mode in ("general", "cmul"):
                p1 = ppool.tile([128, ncols], F32)
                nc.sync.dma_start(out=p1[:], in_=p1_h[:])
            if mode == "cmul":
                p2 = ppool.tile([128, ncols], F32)
                nc.sync.dma_start(out=p2[:], in_=p2_h[:])

            for t in range(t_tiles):
                x = xs[t]
                for q in range(n_groups):
                    g = gpool.tile([128, group_u * b_core], F32)
                    y = ypool.tile([128, group_u * b_core], F32)
                    for u in range(group_u):
                        col = t * k + q * group_u + u
                        sl = slice(u * b_core, (u + 1) * b_core)
                        # s = (a*x + b)^2  (per-partition a, b)
                        nc.scalar.activation(
                            y[:, sl],
                            x[:],
                            mybir.ActivationFunctionType.Square,
                            bias=p1[:, col : col + 1],
                            scale=p0[:, col : col + 1],
                        )
                        # g = exp(-s + lnc)
                        nc.scalar.activation(
                            g[:, sl],
                            y[:, sl],
                            mybir.ActivationFunctionType.Exp,
                            bias=plc[:, col : col + 1],
                            scale=-1.0,
                        )
                        if mode == "cmul":
                            nc.vector.tensor_scalar(
                                g[:, sl],
                                g[:, sl],
                                p2[:, col : col + 1],
                                None,
                                mybir.AluOpType.mult,
                            )
                    nc.sync.dma_start(out=out_h[t, q], in_=g[:])

    _split_multi_waits(nc)
    return nc


# ---------------------------------------------------------------------------
# Host orchestration
# ---------------------------------------------------------------------------

def _prep(x, expansion_mapping, means, vars_, coefs):
    x = np.asarray(x, dtype=np.float32)
    em = np.asarray(expansion_mapping, dtype=np.float32)
    means = np.asarray(means, dtype=np.float32)
    vars_ = np.asarray(vars_, dtype=np.float32)
    coefs = np.asarray(coefs, dtype=np.float32)
    return x, em, means, vars_, coefs


def _numpy_fallback(x, em, means, vars_, coefs):
    xe = x @ em.T
    z = (xe - means) / vars_
    return (np.exp(-0.5 * z * z) * coefs).astype(np.float32)


def _run(inputs, trace=False):
    x, em, means, vars_, coefs = _prep(
        inputs["x"], inputs["expansion_mapping"], inputs["means"],
        inputs["vars_"], inputs["coefs"],
    )
    batch, in_f = x.shape
    out_f = em.shape[0]

    idx = em.argmax(axis=1)
    scale_em = em[np.arange(out_f), idx]
    counts = np.bincount(idx, minlength=in_f)
    k = out_f // in_f
    structured = (
        batch % N_CORES == 0
        and in_f % 128 == 0
        and out_f % in_f == 0
        and np.all(counts == k)
        and k % 2 == 0
        # each row must be one-hot (a single nonzero)
        and np.count_nonzero(em) == out_f
    )
    if not structured:
        return _numpy_fallback(x, em, means, vars_, coefs), None

    b_core = batch // N_CORES
    t_tiles = in_f // 128
    ncols = t_tiles * k

    order = np.argsort(idx, kind="stable")  # device row m -> out feature
    a = scale_em[order] / vars_[order]
    b = -means[order] / vars_[order]
    c = coefs[order]

    fast = bool(np.all(b == 0.0))
    if np.all(c >= 0.0):
        with np.errstate(divide="ignore"):
            lc = np.log(c).astype(np.float32)
        cmul = None
    else:
        lc = np.zeros(out_f, dtype=np.float32)
        cmul = c

    if fast and cmul is None:
        mode = "fast"
        p0 = (-0.5 * a * a).astype(np.float32)
    elif cmul is None:
        mode = "general"
        p0 = (a / np.sqrt(2.0)).astype(np.float32)
        p1 = (b / np.sqrt(2.0)).astype(np.float32)
    else:
        mode = "cmul"
        p0 = (a / np.sqrt(2.0)).astype(np.float32)
        p1 = (b / np.sqrt(2.0)).astype(np.float32)

    def dev_layout(p):
        # full [out_f] ordered by device row m = 16*i + r -> [128, ncols]
        return np.ascontiguousarray(
            p.reshape(t_tiles, 128, k).transpose(1, 0, 2).reshape(128, ncols)
        )

    if mode == "fast":
        if ncols == sum(n for n, _ in PLAN):
            plan = PLAN
        else:
            plan = [
                (2, ("sync", "scalar")[i % 2]) for i in range(ncols // 2)
            ]
        nc = _build_nc_fast(in_f, out_f, b_core, plan)
        group_u = n_groups = None
    else:
        group_u = 4 if k % 4 == 0 else k
        n_groups = k // group_u
        nc = _build_nc(in_f, out_f, b_core, mode, group_u)

    xT = np.ascontiguousarray(x.T)  # [in_f, batch]
    if mode == "fast":
        base = {
            "pp": np.ascontiguousarray(
                np.hstack([dev_layout(p0), dev_layout(lc)])
            )
        }
    else:
        base = {"p0": dev_layout(p0), "plc": dev_layout(lc)}
        base["p1"] = dev_layout(p1)
        if mode == "cmul":
            base["p2"] = dev_layout(cmul)
    in_maps = []
    for kcore in range(N_CORES):
        shard = np.ascontiguousarray(
            xT[:, kcore * b_core : (kcore + 1) * b_core]
        ).reshape(t_tiles, 128, b_core)
        in_maps.append({"xt": shard, **base})

    res = run_bass_kernel_spmd(
        nc, in_maps, core_ids=list(range(N_CORES)), trace=trace
    )

    identity_order = bool(np.array_equal(order, np.arange(out_f)))
    out = np.empty((batch, out_f), dtype=np.float32)
    for kcore in range(N_CORES):
        dev = res.results[kcore]["out"]
        if mode == "fast":
            # [ncols//2, 128, 2*b_core]: block = global unit pair
            if dev.dtype != np.float32:
                dev = dev.astype(np.float32)
            dev = dev.reshape(t_tiles, k // 2, 128, 2, b_core)
        else:
            # [t, q, 128, group_u*b_core]
            dev = dev.reshape(t_tiles, n_groups, 128, group_u, b_core)
        # row m = 16*(128*t + p) + r  with r = (pairing axes 1,3)
        dev = dev.transpose(0, 2, 1, 3, 4).reshape(out_f, b_core)
        sl = slice(kcore * b_core, (kcore + 1) * b_core)
        if identity_order:
            out[sl] = dev.T
        else:
            out[sl.start : sl.stop, order] = dev.T
    return out, res


def kernel(**inputs):
    out, _ = _run(inputs, trace=False)
    return out
